# revision 27
# baseline (speedup 1.0000x reference)
"""DPCL objective (deep-clustering loss) on 8 Trainium2 NeuronCores.

Strategy (pure data parallel, batch dim N=16 -> 2 utterances per core):
For each utterance, the loss only needs the 42x40 weighted Gram matrix

    G = [W*E | wo0 | wo1]^T @ E        (contraction over FT = 154200)

where E is the (FT, 40) embedding, W = diag(magnitude_mix row), and
wo_s = magnitude_mix * onehot_s(argmax(magnitude_ref)).  Because
weights enter the affinity bilinearly, scaling ONE matmul operand by the
un-normalized magnitudes is enough:

    A  = out^T out = G[0:40] / M,   C^T = G[40:42] / M,
    B  = diag(b0, b1)/M,  b_s = sum_k wo_s[k],  M = b0 + b1
    loss_n = (||A||^2 + ||B||^2 - 2||C||^2) / T

Each core accumulates G in PSUM over 1205 chunk matmuls (contraction dim
128 per matmul), plus per-row mask/scale prep on the vector engine.  The
host only slices inputs per core and combines the 16 tiny (42x40) Grams
into the scalar loss.

FT layout: 154200 = 128*1204 + 88.  Rows are assigned partition-major
(partition p owns rows [p*1204, (p+1)*1204)), which makes every DMA
per-partition contiguous without any host-side copies; the 88-row tail is
handled as one extra small-K matmul.
"""

import os
import sys
import numpy as np
from contextlib import ExitStack

sys.path.insert(0, "/opt/trn_rl_repo")

N_FULL = 16
F, T, S, D = 257, 600, 2, 40
FT = F * T                      # 154200
NCORES = 8
NPER = N_FULL // NCORES         # 2 utterances per core
P = 128

# full-size FT decomposition: FT = P*CPP + TAIL
CPP = FT // P                   # 1204 columns per partition (main part)
MAIN = P * CPP                  # 154112
TAIL = FT - MAIN                # 88
CB = 86                         # chunks per group
NGROUPS = CPP // CB             # 14

# matmul operand dtype / transfer strategy:
#   "f32"      - fp32 matmuls (4 cyc/row), fp32 DMA
#   "bf16"     - bf16 matmuls, cast-during-DMA (SWDGE), fp32 HBM reads
#   "bf16host" - bf16 matmuls, embedding pre-cast on host (halves HBM reads)
#   "perm"     - bf16host + host-permuted [P, D, c] layout (packed 2x DVE
#                weighted-copy) + PE column-tiling (2 concurrent chunks)
MODE = os.environ.get("DPCL_MODE", "tile2")
EW = 172                        # E-tile chunk width (2 groups of CB)
NG_POOL = int(os.environ.get("DPCL_NGPOOL", "0"))      # WE groups on GpSimd
PREP_POOL = os.environ.get("DPCL_PREPPOOL", "1") == "1"  # mask prep on GpSimd

LAST_EXEC_NS = None

_prog_cache = {}


def _build_program(nper, cpp, cb, ngroups, tail, mode):
    import concourse.bass as bass
    import concourse.bacc as bacc
    import concourse.tile as tile
    from concourse import mybir

    f32 = mybir.dt.float32
    dmm = f32 if mode == "f32" else mybir.dt.bfloat16
    ft = P * cpp + tail
    main = P * cpp
    assert ngroups * cb == cpp

    nc = bacc.Bacc(
        "TRN2", target_bir_lowering=False, debug=False, num_devices=NCORES
    )
    emb_dt = dmm if mode == "bf16host" else f32
    emb = nc.declare_dram_parameter("emb", [nper, ft, D], emb_dt, isOutput=False)
    mm = nc.declare_dram_parameter("mm", [nper, ft], f32, isOutput=False)
    mref = nc.declare_dram_parameter("mref", [nper, ft, S], f32, isOutput=False)
    g_out = nc.declare_dram_parameter("g_out", [nper, D + S, D], f32, isOutput=True)
    b_out = nc.declare_dram_parameter("b_out", [nper, P, S], f32, isOutput=True)

    # engine used for the big E loads (SWDGE supports dtype-cast during DMA)
    if mode == "bf16":
        e_dma = lambda out, in_: nc.gpsimd.dma_start(out=out, in_=in_)
    else:
        e_dma = lambda out, in_: nc.sync.dma_start(out=out, in_=in_)
    # in bf16 (cast-DMA) mode GpSimd is busy generating descriptors; otherwise
    # split the big weighted-copy work between DVE and GpSimd
    split_we = mode != "bf16"

    with tile.TileContext(nc) as tc, ExitStack() as ctx:
        wpool = ctx.enter_context(tc.tile_pool(name="wpool", bufs=2))
        epool = ctx.enter_context(tc.tile_pool(name="epool", bufs=3))
        lpool = ctx.enter_context(tc.tile_pool(name="lpool", bufs=3))
        spool = ctx.enter_context(tc.tile_pool(name="spool", bufs=2))
        psum = ctx.enter_context(tc.tile_pool(name="psum", bufs=2, space="PSUM"))

        for u in range(nper):
            # ---- per-row weight / mask prep (all [128, cpp]) ----
            w_t = wpool.tile([P, cpp], f32, tag="w")
            nc.sync.dma_start(
                out=w_t[:], in_=mm[u, 0:main].rearrange("(p c) -> p c", p=P)
            )
            mr_t = wpool.tile([P, cpp * S], f32, tag="mr")
            nc.sync.dma_start(
                out=mr_t[:],
                in_=mref[u, 0:main, :].rearrange("(p c) s -> p (c s)", p=P),
            )
            mr3 = mr_t[:].rearrange("p (c s) -> p c s", s=S)
            mask_t = wpool.tile([P, cpp], f32, tag="mask")
            # mask = 1.0 where speaker-1 magnitude wins the argmax
            nc.vector.tensor_tensor(
                mask_t[:], mr3[:, :, 1], mr3[:, :, 0], mybir.AluOpType.is_gt
            )
            wo_t = wpool.tile([P, S * cpp], f32, tag="wo")  # [wo0 | wo1]
            nc.vector.tensor_mul(wo_t[:, cpp : 2 * cpp], w_t[:], mask_t[:])
            nc.vector.tensor_sub(wo_t[:, 0:cpp], w_t[:], wo_t[:, cpp : 2 * cpp])
            wo3 = wo_t[:].rearrange("p (s c) -> p c s", s=S)

            wored = spool.tile([P, S], f32, tag="wored")
            nc.vector.tensor_reduce(
                wored[:],
                wo_t[:].rearrange("p (s c) -> p s c", s=S),
                mybir.AxisListType.X,
                mybir.AluOpType.add,
            )

            # ---- tail prep ([tail, *]) ----
            wtl = spool.tile([P, 1], f32, tag="wtl")
            nc.sync.dma_start(out=wtl[0:tail, :], in_=mm[u, main:ft].unsqueeze(1))
            mrtl = spool.tile([P, S], f32, tag="mrtl")
            nc.sync.dma_start(out=mrtl[0:tail, :], in_=mref[u, main:ft, :])
            masktl = spool.tile([P, 1], f32, tag="masktl")
            nc.vector.tensor_tensor(
                masktl[0:tail, :],
                mrtl[0:tail, 1:2],
                mrtl[0:tail, 0:1],
                mybir.AluOpType.is_gt,
            )
            wotl = spool.tile([P, S], f32, tag="wotl")
            nc.vector.tensor_mul(wotl[0:tail, 1:2], wtl[0:tail, :], masktl[0:tail, :])
            nc.vector.tensor_sub(wotl[0:tail, 0:1], wtl[0:tail, :], wotl[0:tail, 1:2])
            nc.vector.tensor_add(wored[0:tail, :], wored[0:tail, :], wotl[0:tail, :])
            nc.sync.dma_start(out=b_out[u, :, :], in_=wored[:])

            # ---- Gram accumulation ----
            gp = psum.tile([D + S, D], f32, tag="g")
            e_main = emb[u, 0:main, :].rearrange("(p c) d -> p c d", p=P)
            for g in range(ngroups):
                et = epool.tile([P, cb * D], dmm, tag="e")
                e3 = et[:].rearrange("p (c d) -> p c d", d=D)
                e_dma(e3[:], e_main[:, g * cb : (g + 1) * cb, :])

                lt = lpool.tile([P, cb * (D + S)], dmm, tag="l")
                l3 = lt[:].rearrange("p (c e) -> p c e", e=D + S)
                # weighted copy of E into the stationary operand
                wslice = w_t[:, g * cb : (g + 1) * cb].unsqueeze(2).broadcast_to(
                    [P, cb, D]
                )
                weng = nc.gpsimd if (split_we and g % 2 == 1) else nc.vector
                weng.tensor_mul(l3[:, :, 0:D], e3[:], wslice)
                # masked-weight columns (wo0, wo1)
                weng.tensor_copy(
                    l3[:, :, D : D + S], wo3[:, g * cb : (g + 1) * cb, :]
                )
                for c in range(cb):
                    nc.tensor.matmul(
                        gp[:],
                        lt[:, c * (D + S) : (c + 1) * (D + S)],
                        et[:, c * D : (c + 1) * D],
                        start=(g == 0 and c == 0),
                        stop=False,
                    )

            # tail chunk (contraction dim = tail)
            etl = spool.tile([P, D], dmm, tag="etl")
            e_dma(etl[0:tail, :], emb[u, main:ft, :])
            ltl = spool.tile([P, D + S], dmm, tag="ltl")
            nc.vector.tensor_mul(
                ltl[0:tail, 0:D],
                etl[0:tail, :],
                wtl[0:tail, :].broadcast_to([tail, D]),
            )
            nc.vector.tensor_copy(ltl[0:tail, D : D + S], wotl[0:tail, :])
            nc.tensor.matmul(
                gp[:], ltl[0:tail, :], etl[0:tail, :], start=False, stop=True
            )

            gsb = spool.tile([D + S, D], f32, tag="gsb")
            nc.scalar.activation(gsb[:], gp[:], mybir.ActivationFunctionType.Copy)
            nc.sync.dma_start(out=g_out[u, :, :], in_=gsb[:])

    nc.compile()
    return nc


def _build_perm(nper, cpp, ew, cb, tail):
    """Permuted-layout bf16 build: E arrives as [nper, P, D, cpp] so the
    weighted copy hits DVE's packed 2x mode, and chunks alternate between
    two PE column-tile positions (the 42-col stationary only uses a third
    of the array)."""
    import concourse.bacc as bacc
    import concourse.tile as tile
    from concourse import mybir

    f32 = mybir.dt.float32
    bf16 = mybir.dt.bfloat16
    ft = P * cpp + tail
    main = P * cpp
    ntiles = cpp // ew
    gpe = ew // cb
    assert ntiles * ew == cpp and gpe * cb == ew and cb % 2 == 0

    nc = bacc.Bacc(
        "TRN2", target_bir_lowering=False, debug=False, num_devices=NCORES
    )
    emb_p = nc.declare_dram_parameter("emb_p", [nper, P, D, cpp], bf16, isOutput=False)
    emb_t = nc.declare_dram_parameter("emb_t", [nper, tail, D], bf16, isOutput=False)
    mm = nc.declare_dram_parameter("mm", [nper, ft], f32, isOutput=False)
    mref = nc.declare_dram_parameter("mref", [nper, ft, S], f32, isOutput=False)
    g_out = nc.declare_dram_parameter(
        "g_out", [nper, 2, D + S, D], f32, isOutput=True
    )
    b_out = nc.declare_dram_parameter("b_out", [nper, P, S], f32, isOutput=True)

    with tile.TileContext(nc) as tc, ExitStack() as ctx:
        wpool = ctx.enter_context(tc.tile_pool(name="wpool", bufs=2))
        epool = ctx.enter_context(tc.tile_pool(name="epool", bufs=3))
        lpool = ctx.enter_context(tc.tile_pool(name="lpool", bufs=3))
        spool = ctx.enter_context(tc.tile_pool(name="spool", bufs=2))
        psum = ctx.enter_context(tc.tile_pool(name="psum", bufs=2, space="PSUM"))

        for u in range(nper):
            # ---- per-row weight / mask prep (all [128, cpp], fp32) ----
            w_t = wpool.tile([P, cpp], f32, tag="w")
            nc.sync.dma_start(
                out=w_t[:], in_=mm[u, 0:main].rearrange("(p c) -> p c", p=P)
            )
            mr_t = wpool.tile([P, cpp * S], f32, tag="mr")
            nc.sync.dma_start(
                out=mr_t[:],
                in_=mref[u, 0:main, :].rearrange("(p c) s -> p (c s)", p=P),
            )
            mr3 = mr_t[:].rearrange("p (c s) -> p c s", s=S)
            mask_t = wpool.tile([P, cpp], f32, tag="mask")
            nc.vector.tensor_tensor(
                mask_t[:], mr3[:, :, 1], mr3[:, :, 0], mybir.AluOpType.is_gt
            )
            wo_t = wpool.tile([P, S * cpp], f32, tag="wo")  # [wo0 | wo1]
            nc.vector.tensor_mul(wo_t[:, cpp : 2 * cpp], w_t[:], mask_t[:])
            nc.vector.tensor_sub(wo_t[:, 0:cpp], w_t[:], wo_t[:, cpp : 2 * cpp])
            wo_sc = wo_t[:].rearrange("p (s c) -> p s c", s=S)
            w_bf = wpool.tile([P, cpp], bf16, tag="wbf")
            nc.vector.tensor_copy(w_bf[:], w_t[:])

            wored = spool.tile([P, S], f32, tag="wored")
            nc.vector.tensor_reduce(
                wored[:],
                wo_t[:].rearrange("p (s c) -> p s c", s=S),
                mybir.AxisListType.X,
                mybir.AluOpType.add,
            )

            # ---- tail prep ----
            wtl = spool.tile([P, 1], f32, tag="wtl")
            nc.sync.dma_start(out=wtl[0:tail, :], in_=mm[u, main:ft].unsqueeze(1))
            mrtl = spool.tile([P, S], f32, tag="mrtl")
            nc.sync.dma_start(out=mrtl[0:tail, :], in_=mref[u, main:ft, :])
            masktl = spool.tile([P, 1], f32, tag="masktl")
            nc.vector.tensor_tensor(
                masktl[0:tail, :],
                mrtl[0:tail, 1:2],
                mrtl[0:tail, 0:1],
                mybir.AluOpType.is_gt,
            )
            wotl = spool.tile([P, S], f32, tag="wotl")
            nc.vector.tensor_mul(wotl[0:tail, 1:2], wtl[0:tail, :], masktl[0:tail, :])
            nc.vector.tensor_sub(wotl[0:tail, 0:1], wtl[0:tail, :], wotl[0:tail, 1:2])
            nc.vector.tensor_add(wored[0:tail, :], wored[0:tail, :], wotl[0:tail, :])
            nc.sync.dma_start(out=b_out[u, :, :], in_=wored[:])

            # ---- Gram accumulation, two column-tile positions ----
            gp = psum.tile([P, D], f32, tag="g")
            started = [False, False]
            for t in range(ntiles):
                et = epool.tile([P, D * ew], bf16, tag="e")
                e3 = et[:].rearrange("p (d c) -> p d c", c=ew)
                nc.sync.dma_start(
                    out=e3[:], in_=emb_p[u, :, :, t * ew : (t + 1) * ew]
                )
                for gc in range(gpe):
                    co = gc * cb
                    lt = lpool.tile([P, cb * (D + S)], bf16, tag="l")
                    l3 = lt[:].rearrange("p (e c) -> p e c", c=cb)
                    wsl = (
                        w_bf[:, t * ew + co : t * ew + co + cb]
                        .unsqueeze(1)
                        .broadcast_to([P, D, cb])
                    )
                    nc.vector.tensor_mul(l3[:, 0:D, :], e3[:, :, co : co + cb], wsl)
                    nc.vector.tensor_copy(
                        l3[:, D : D + S, :],
                        wo_sc[:, :, t * ew + co : t * ew + co + cb],
                    )
                    for c in range(cb):
                        k = t * ew + co + c
                        par = k % 2
                        pb = 64 * par
                        st = not started[par]
                        started[par] = True
                        nc.tensor.matmul(
                            gp[pb : pb + D + S, :],
                            l3[:, :, c : c + 1],
                            e3[:, :, co + c : co + c + 1],
                            start=st,
                            stop=(par == 1 and k == cpp - 1),
                            tile_position=(0, pb),
                            skip_group_check=True,
                        )

            # tail chunk -> position 0 accumulator, closes its group
            etl = spool.tile([P, D], bf16, tag="etl")
            nc.sync.dma_start(out=etl[0:tail, :], in_=emb_t[u, :, :])
            ltl = spool.tile([P, D + S], bf16, tag="ltl")
            nc.vector.tensor_mul(
                ltl[0:tail, 0:D],
                etl[0:tail, :],
                wtl[0:tail, :].broadcast_to([tail, D]),
            )
            nc.vector.tensor_copy(ltl[0:tail, D : D + S], wotl[0:tail, :])
            nc.tensor.matmul(
                gp[0 : D + S, :],
                ltl[0:tail, :],
                etl[0:tail, :],
                start=False,
                stop=True,
                tile_position=(0, 0),
                skip_group_check=True,
            )

            gsb = spool.tile([P, D], f32, tag="gsb")
            nc.scalar.activation(
                gsb[0 : D + S, :], gp[0 : D + S, :], mybir.ActivationFunctionType.Copy
            )
            nc.scalar.activation(
                gsb[64 : 64 + D + S, :],
                gp[64 : 64 + D + S, :],
                mybir.ActivationFunctionType.Copy,
            )
            nc.sync.dma_start(out=g_out[u, 0, :, :], in_=gsb[0 : D + S, :])
            nc.sync.dma_start(out=g_out[u, 1, :, :], in_=gsb[64 : 64 + D + S, :])

    nc.compile()
    return nc


def _build_tile2(nper, cpp, ew, cb, tail, ng_pool=0, prep_pool=True):
    """Contiguous (c,d) layouts for all PE operands + 2-way PE column
    tiling + DVE/GpSimd split of the weighted copy + dual HWDGE rings."""
    import concourse.bacc as bacc
    import concourse.tile as tile
    from concourse import mybir

    f32 = mybir.dt.float32
    bf16 = mybir.dt.bfloat16
    ft = P * cpp + tail
    main = P * cpp
    ntiles = cpp // ew
    gpe = ew // cb
    assert ntiles * ew == cpp and gpe * cb == cb * gpe and gpe * cb == ew

    nc = bacc.Bacc(
        "TRN2", target_bir_lowering=False, debug=False, num_devices=NCORES
    )
    emb = nc.declare_dram_parameter("emb", [nper, ft, D], bf16, isOutput=False)
    mm = nc.declare_dram_parameter("mm", [nper, ft], f32, isOutput=False)
    mref = nc.declare_dram_parameter("mref", [nper, ft, S], f32, isOutput=False)
    g_out = nc.declare_dram_parameter(
        "g_out", [nper, 2, D + S, D], f32, isOutput=True
    )
    b_out = nc.declare_dram_parameter("b_out", [nper, P, S], f32, isOutput=True)

    total_groups = nper * ntiles * gpe

    with tile.TileContext(nc) as tc, ExitStack() as ctx:
        wpool = ctx.enter_context(tc.tile_pool(name="wpool", bufs=2))
        epool = ctx.enter_context(tc.tile_pool(name="epool", bufs=5))
        lpool = ctx.enter_context(tc.tile_pool(name="lpool", bufs=4))
        spool = ctx.enter_context(tc.tile_pool(name="spool", bufs=2))
        psum = ctx.enter_context(tc.tile_pool(name="psum", bufs=2, space="PSUM"))

        gi = 0  # global group index for the DVE/GpSimd split
        for u in range(nper):
            # ---- per-row weight / mask prep (fp32 [128, cpp]) ----
            w_t = wpool.tile([P, cpp], f32, tag="w")
            nc.sync.dma_start(
                out=w_t[:], in_=mm[u, 0:main].rearrange("(p c) -> p c", p=P)
            )
            mr_t = wpool.tile([P, cpp * S], f32, tag="mr")
            nc.scalar.dma_start(
                out=mr_t[:],
                in_=mref[u, 0:main, :].rearrange("(p c) s -> p (c s)", p=P),
            )
            mr3 = mr_t[:].rearrange("p (c s) -> p c s", s=S)
            peng = nc.gpsimd if prep_pool else nc.vector
            mask_t = wpool.tile([P, cpp], f32, tag="mask")
            nc.vector.tensor_tensor(
                mask_t[:], mr3[:, :, 1], mr3[:, :, 0], mybir.AluOpType.is_gt
            )
            wo_t = wpool.tile([P, S * cpp], f32, tag="wo")  # [wo0 | wo1]
            peng.tensor_mul(wo_t[:, cpp : 2 * cpp], w_t[:], mask_t[:])
            peng.tensor_sub(wo_t[:, 0:cpp], w_t[:], wo_t[:, cpp : 2 * cpp])
            wo_sc = wo_t[:].rearrange("p (s c) -> p s c", s=S)

            wored = spool.tile([P, S], f32, tag="wored")
            nc.vector.tensor_reduce(
                wored[:],
                wo_t[:].rearrange("p (s c) -> p s c", s=S),
                mybir.AxisListType.X,
                mybir.AluOpType.add,
            )

            # ---- tail prep ----
            wtl = spool.tile([P, 1], f32, tag="wtl")
            nc.sync.dma_start(out=wtl[0:tail, :], in_=mm[u, main:ft].unsqueeze(1))
            mrtl = spool.tile([P, S], f32, tag="mrtl")
            nc.sync.dma_start(out=mrtl[0:tail, :], in_=mref[u, main:ft, :])
            masktl = spool.tile([P, 1], f32, tag="masktl")
            nc.vector.tensor_tensor(
                masktl[0:tail, :],
                mrtl[0:tail, 1:2],
                mrtl[0:tail, 0:1],
                mybir.AluOpType.is_gt,
            )
            wotl = spool.tile([P, S], f32, tag="wotl")
            nc.vector.tensor_mul(wotl[0:tail, 1:2], wtl[0:tail, :], masktl[0:tail, :])
            nc.vector.tensor_sub(wotl[0:tail, 0:1], wtl[0:tail, :], wotl[0:tail, 1:2])
            nc.vector.tensor_add(wored[0:tail, :], wored[0:tail, :], wotl[0:tail, :])
            nc.sync.dma_start(out=b_out[u, :, :], in_=wored[:])

            # ---- Gram accumulation ----
            gp = psum.tile([P, D], f32, tag="g")
            started = [False, False]
            e_main = emb[u, 0:main, :].rearrange("(p c) d -> p c d", p=P)
            for t in range(ntiles):
                et = epool.tile([P, ew * D], bf16, tag="e")
                e3 = et[:].rearrange("p (c d) -> p c d", d=D)
                # SWDGE: ~1us Q7 descriptor emission, then fire-and-forget --
                # HWDGE dma_start blocks its issuing engine for the whole
                # transfer, which serializes the big loads
                nc.gpsimd.dma_start(out=e3[:], in_=e_main[:, t * ew : (t + 1) * ew, :])
                for gc in range(gpe):
                    co = gc * cb
                    lt = lpool.tile([P, cb * (D + S)], bf16, tag="l")
                    l3 = lt[:].rearrange("p (c e) -> p c e", e=D + S)
                    wsl = (
                        w_t[:, t * ew + co : t * ew + co + cb]
                        .unsqueeze(2)
                        .broadcast_to([P, cb, D])
                    )
                    weng = (
                        nc.gpsimd
                        if ng_pool and (gi % (total_groups // max(ng_pool, 1))) == 0
                        else nc.vector
                    )
                    gi += 1
                    weng.tensor_mul(l3[:, :, 0:D], e3[:, co : co + cb, :], wsl)
                    nc.scalar.activation(
                        l3[:, :, D : D + S],
                        wo_sc[:, :, t * ew + co : t * ew + co + cb].transpose(
                            [0, 2, 1]
                        ),
                        mybir.ActivationFunctionType.Copy,
                    )
                    for c in range(cb):
                        k = t * ew + co + c
                        par = k % 2
                        pb = 64 * par
                        st = not started[par]
                        started[par] = True
                        nc.tensor.matmul(
                            gp[pb : pb + D + S, :],
                            lt[:, c * (D + S) : (c + 1) * (D + S)],
                            et[:, c * D + co * D : (c + 1) * D + co * D],
                            start=st,
                            stop=(par == 1 and k == cpp - 1),
                            tile_position=(0, pb),
                            skip_group_check=True,
                        )

            # tail chunk -> position 0 accumulator, closes its group
            etl = spool.tile([P, D], bf16, tag="etl")
            nc.sync.dma_start(out=etl[0:tail, :], in_=emb[u, main:ft, :])
            ltl = spool.tile([P, D + S], bf16, tag="ltl")
            nc.vector.tensor_mul(
                ltl[0:tail, 0:D],
                etl[0:tail, :],
                wtl[0:tail, :].broadcast_to([tail, D]),
            )
            nc.vector.tensor_copy(ltl[0:tail, D : D + S], wotl[0:tail, :])
            nc.tensor.matmul(
                gp[0 : D + S, :],
                ltl[0:tail, :],
                etl[0:tail, :],
                start=False,
                stop=True,
                tile_position=(0, 0),
                skip_group_check=True,
            )

            gsb = spool.tile([P, D], f32, tag="gsb")
            nc.scalar.activation(
                gsb[0 : D + S, :], gp[0 : D + S, :], mybir.ActivationFunctionType.Copy
            )
            nc.scalar.activation(
                gsb[64 : 64 + D + S, :],
                gp[64 : 64 + D + S, :],
                mybir.ActivationFunctionType.Copy,
            )
            nc.sync.dma_start(out=g_out[u, 0, :, :], in_=gsb[0 : D + S, :])
            nc.sync.dma_start(out=g_out[u, 1, :, :], in_=gsb[64 : 64 + D + S, :])

    nc.compile()
    return nc


def _get_program(key):
    if key not in _prog_cache:
        if key[-1] == "perm":
            _prog_cache[key] = _build_perm(*key[:-1])
        elif key[-1] == "tile2":
            _prog_cache[key] = _build_tile2(
                *key[:-1], ng_pool=NG_POOL, prep_pool=PREP_POOL
            )
        else:
            _prog_cache[key] = _build_program(*key)
    return _prog_cache[key]


def _finish_host(g_all, b_all):
    """g_all: [N, 42, 40] (or [N, 2, 42, 40]), b_all: [N, P, 2] -> loss."""
    if g_all.ndim == 4:
        g_all = g_all.sum(axis=1, dtype=np.float64)
    g = g_all.astype(np.float64)
    b = b_all.astype(np.float64).sum(axis=1)  # [N, 2]
    a2 = (g[:, 0:D, :] ** 2).sum(axis=(1, 2))
    c2 = (g[:, D : D + S, :] ** 2).sum(axis=(1, 2))
    b2 = (b**2).sum(axis=1)
    m = b.sum(axis=1)
    loss = (a2 + b2 - 2.0 * c2) / (m * m * T)
    return np.float32(loss.mean())


def _install_trace_shim():
    """Provide the antenv.axon_hooks module bass_utils expects for NTFF
    profiling under axon (this image's antenv lacks it)."""
    import sys as _sys
    import types

    if "antenv.axon_hooks" in _sys.modules:
        return
    try:
        from trn_agent_boot.trn_boot import _ntff_profile_via_ctypes

        hook = _ntff_profile_via_ctypes("/opt/axon/libaxon_pjrt.so")
    except Exception:
        hook = None
    mod = types.ModuleType("antenv.axon_hooks")
    mod.get_axon_ntff_profile_hook = lambda: hook
    mod.set_axon_ntff_profile_hook = lambda h: None
    _sys.modules["antenv.axon_hooks"] = mod


def kernel(embedding, magnitude_ref, magnitude_mix):
    from concourse.bass_utils import run_bass_kernel_spmd

    global LAST_EXEC_NS
    mref = np.ascontiguousarray(magnitude_ref, dtype=np.float32).reshape(N_FULL, FT, S)
    mm = np.ascontiguousarray(magnitude_mix, dtype=np.float32).reshape(N_FULL, FT)
    core_ids = list(range(NCORES))

    if MODE == "perm":
        import ml_dtypes

        emb32 = np.ascontiguousarray(embedding, dtype=np.float32)
        emb_p = (
            emb32[:, :MAIN, :]
            .reshape(N_FULL, P, CPP, D)
            .transpose(0, 1, 3, 2)
            .astype(ml_dtypes.bfloat16)
        )
        emb_t = emb32[:, MAIN:, :].astype(ml_dtypes.bfloat16)
        nc = _get_program((NPER, CPP, EW, CB, TAIL, "perm"))
        in_maps = [
            {
                "emb_p": emb_p[i * NPER : (i + 1) * NPER],
                "emb_t": emb_t[i * NPER : (i + 1) * NPER],
                "mm": mm[i * NPER : (i + 1) * NPER],
                "mref": mref[i * NPER : (i + 1) * NPER],
            }
            for i in core_ids
        ]
    elif MODE == "tile2":
        import ml_dtypes

        emb = np.ascontiguousarray(embedding).astype(ml_dtypes.bfloat16)
        nc = _get_program((NPER, CPP, EW, CB, TAIL, "tile2"))
        in_maps = [
            {
                "emb": emb[i * NPER : (i + 1) * NPER],
                "mm": mm[i * NPER : (i + 1) * NPER],
                "mref": mref[i * NPER : (i + 1) * NPER],
            }
            for i in core_ids
        ]
    else:
        if MODE == "bf16host":
            import ml_dtypes

            emb = np.ascontiguousarray(embedding).astype(ml_dtypes.bfloat16)
        else:
            emb = np.ascontiguousarray(embedding, dtype=np.float32)
        nc = _get_program((NPER, CPP, CB, NGROUPS, TAIL, MODE))
        in_maps = [
            {
                "emb": emb[i * NPER : (i + 1) * NPER],
                "mm": mm[i * NPER : (i + 1) * NPER],
                "mref": mref[i * NPER : (i + 1) * NPER],
            }
            for i in core_ids
        ]
    trace = os.environ.get("DPCL_TRACE", "0") == "1"
    if trace:
        _install_trace_shim()
    res = run_bass_kernel_spmd(nc, in_maps, core_ids, trace=trace)
    LAST_EXEC_NS = res.exec_time_ns

    g_all = np.concatenate([r["g_out"] for r in res.results], axis=0)
    b_all = np.concatenate([r["b_out"] for r in res.results], axis=0)
    return _finish_host(g_all, b_all)
